# revision 6
# baseline (speedup 1.0000x reference)
"""Banded cross-attention (sparse_attention) TRN2 Bass kernel.

Problem: q1 = y1^T Wq, k2 = y2^T Wk, v2 = y2^T Wv  (y: [B,C,T], W: [C,U])
         e[b,i,j] = q1[b,i,:].k2[b,j,:]/sqrt(U) masked to band j-i in [-127,128]
         a = softmax(e, axis=-1);  out = einsum('bij,bju->bui', a, v2)
         returns (out [B,U,T], a [B,T,T])

Sharding: 8 cores = (B=4) x (2 halves of T). Each core handles 2048 query rows.

Device per core (SPMD, identical program):
  - projections: qT [64,2048], kT [64,2304] (with 128-halo, zero-padded at
    global edges), v [128,18,64] (18 j-chunks of 128).
  - per 128-row block h (16 blocks): score window [128 x 384]
    (window cols j = t0-128+128h .. +384). PSUM preloaded with additive band
    mask via identity-matmul, then q.k accumulates on top. ACT exp computes
    s = exp(e+mask) and its row-sum (denominator) via accum_out for free.
    s block DMA'd out (unnormalized attention numerators).
  - 3 PE transposes per block build sT [j,i] chunks; out_raw[u,i] += v^T sT
    accumulated over 6 j-chunks per 512-row group.
  - ships: a_cmp [16,128,384], out_raw [64,2048], den [128,16]. All
    unnormalized; host divides by denominators.

Host: corrects denominators at global edges (each in-band slot with padded
k==0 contributed exactly exp(0)=1), normalizes, scatters band into dense a.
"""

import numpy as np
from contextlib import ExitStack

import concourse.bass as bass
import concourse.bacc as bacc
import concourse.tile as tile
from concourse import mybir
from concourse.bass_utils import run_bass_kernel_spmd

F32 = mybir.dt.float32
F32R = mybir.dt.float32r
AF = mybir.ActivationFunctionType
ts = bass.ts

B, C, T, U = 4, 256, 4096, 64
W = 256
NCORES = 8
SH = T // 2             # 2048 rows per core
HALO = 128
KW = SH + 2 * HALO      # 2304 kT columns per core
NBLK = SH // 128        # 16 blocks
NGRP = SH // 512        # 4 groups
WIN = 384               # score window width per block
NEG = np.float32(-1.0e38)

MM_DT = F32R            # matmul dtype for q/k/score/out matmuls

# out-matmul emission: (chunk c, col0, col1, start, stop). Chunk c covers
# sT columns written by blocks {c-1,c,c+1} & [0,4). start=True ops together
# first-touch every PSUM column exactly once.
EMITS = [
    (1, 0, 384, True, False),
    (4, 384, 512, False, False),
    (2, 128, 512, False, False),
    (-1, 0, 128, False, False),
    (0, 0, 256, False, False),
    (3, 256, 512, False, True),
]

_PROG = None


def _body(ctx: ExitStack, tc: "tile.TileContext", aps: dict):
    nc = tc.nc

    consts = ctx.enter_context(tc.tile_pool(name="consts", bufs=1))
    proj_sb = ctx.enter_context(tc.tile_pool(name="proj_sb", bufs=1))
    ypool = ctx.enter_context(tc.tile_pool(name="ypool", bufs=3))
    spool = ctx.enter_context(tc.tile_pool(name="spool", bufs=4))
    stpool = ctx.enter_context(tc.tile_pool(name="stpool", bufs=2))
    dpool = ctx.enter_context(tc.tile_pool(name="dpool", bufs=1))

    mb = consts.tile([128, WIN], MM_DT)
    nc.sync.dma_start(out=mb, in_=aps["mb"])
    ident = consts.tile([128, 128], MM_DT)
    nc.sync.dma_start(out=ident, in_=aps["ident"])
    wq_sb = consts.tile([128, 2, U], MM_DT)
    nc.sync.dma_start(out=wq_sb, in_=aps["wq"].rearrange("(a p) u -> p a u", p=128))
    wk_sb = consts.tile([128, 2, U], MM_DT)
    nc.sync.dma_start(out=wk_sb, in_=aps["wk"].rearrange("(a p) u -> p a u", p=128))
    wv_sb = consts.tile([128, 2, U], MM_DT)
    nc.sync.dma_start(out=wv_sb, in_=aps["wv"].rearrange("(a p) u -> p a u", p=128))

    qT = proj_sb.tile([64, SH], MM_DT)
    kT = proj_sb.tile([64, KW], MM_DT)
    vsb = proj_sb.tile([128, KW // 128, U], MM_DT)
    den_sb = dpool.tile([128, NBLK], F32)

    y1r = aps["y1s"].rearrange("(a p) t -> p a t", p=128)
    y2r = aps["y2s"].rearrange("(a p) t -> p a t", p=128)

    with tc.tile_pool(name="pps", bufs=2, space="PSUM") as pps:
        # q projection: out[u, t-tile] = Wq^T y1
        for it in range(SH // 512):
            yt = ypool.tile([128, 2, 512], MM_DT, tag="yt", name="yt")
            nc.sync.dma_start(out=yt, in_=y1r[:, :, ts(it, 512)])
            ps = pps.tile([64, 512], F32, tag="pq", name="pq")
            nc.tensor.matmul(ps, wq_sb[:, 0],
                             yt[:, 0], start=True, stop=False)
            nc.tensor.matmul(ps, wq_sb[:, 1],
                             yt[:, 1], start=False, stop=True)
            if it % 2 == 0:
                nc.scalar.copy(qT[:, ts(it, 512)], ps)
            else:
                nc.vector.tensor_copy(qT[:, ts(it, 512)], ps)
        # k projection (+ v chunks reuse the same y2 tiles)
        off = 0
        while off < KW:
            n = min(512, KW - off)
            yt = ypool.tile([128, 2, 512], MM_DT, tag="yt", name="yt")
            nc.sync.dma_start(out=yt[:, :, 0:n], in_=y2r[:, :, off:off + n])
            ps = pps.tile([64, 512], F32, tag="pq", name="pq")
            nc.tensor.matmul(ps[:, 0:n], wk_sb[:, 0],
                             yt[:, 0, 0:n], start=True, stop=False)
            nc.tensor.matmul(ps[:, 0:n], wk_sb[:, 1],
                             yt[:, 1, 0:n], start=False, stop=True)
            if (off // 512) % 2 == 0:
                nc.scalar.copy(kT[:, off:off + n], ps[:, 0:n])
            else:
                nc.vector.tensor_copy(kT[:, off:off + n], ps[:, 0:n])
            # v chunks from this y2 tile: v[t-chunk, u] = y2^T Wv
            for cc in range(n // 128):
                ch = off // 128 + cc
                pv = pps.tile([128, U], F32, tag="pv", name="pv")
                nc.tensor.matmul(pv, yt[:, 0, ts(cc, 128)], wv_sb[:, 0],
                                 start=True, stop=False)
                nc.tensor.matmul(pv, yt[:, 1, ts(cc, 128)], wv_sb[:, 1],
                                 start=False, stop=True)
                nc.vector.tensor_copy(vsb[:, ch], pv)
            off += n

    eps_pool = ctx.enter_context(tc.tile_pool(name="eps_pool", bufs=2, space="PSUM"))
    trp_pool = ctx.enter_context(tc.tile_pool(name="trp_pool", bufs=2, space="PSUM"))
    ops_pool = ctx.enter_context(tc.tile_pool(name="ops_pool", bufs=2, space="PSUM"))

    for g in range(NGRP):
        sTs = [stpool.tile([128, 512], MM_DT, tag=f"sT{c}", name=f"sT{c}")
               for c in range(6)]
        for hh in range(4):
            h = g * 4 + hh
            eps = eps_pool.tile([128, WIN], F32, tag="eps", name="eps")
            # preload additive band mask, then accumulate scores on top
            nc.tensor.matmul(eps, ident, mb,
                             start=True, stop=False)
            nc.tensor.matmul(eps, qT[:, ts(h, 128)],
                             kT[:, 128 * h:128 * h + WIN],
                             start=False, stop=True)
            s_sb = spool.tile([128, WIN], F32, tag="s", name="s_sb")
            nc.scalar.activation(s_sb, eps, AF.Exp,
                                 accum_out=den_sb[:, h:h + 1])
            nc.sync.dma_start(out=aps["a_cmp"][h], in_=s_sb)
            trp = trp_pool.tile([128, WIN], F32, tag="trp", name="trp")
            for q3 in range(3):
                nc.tensor.transpose(trp[:, ts(q3, 128)], s_sb[:, ts(q3, 128)],
                                    ident.bitcast(F32))
            for q3 in range(3):
                c = hh + q3 - 1
                nc.vector.tensor_copy(sTs[c + 1][:, ts(hh, 128)],
                                      trp[:, ts(q3, 128)])
        op = ops_pool.tile([U, 512], F32, tag="op", name="op")
        for (c, c0, c1, st, sp) in EMITS:
            vi = 4 * g + c + 1
            nc.tensor.matmul(op[:, c0:c1], vsb[:, vi],
                             sTs[c + 1][:, c0:c1],
                             start=st, stop=sp)
        osb = spool.tile([U, 512], F32, tag="osb", name="osb")
        nc.scalar.copy(osb, op)
        nc.sync.dma_start(out=aps["out_raw"][:, ts(g, 512)], in_=osb)

    nc.sync.dma_start(out=aps["den"], in_=den_sb)


def _build():
    nc = bacc.Bacc("TRN2", target_bir_lowering=False, debug=False,
                   num_devices=NCORES)
    aps = {}
    for name, shape in [("y1s", (C, SH)), ("y2s", (C, KW)), ("wq", (C, U)),
                        ("wk", (C, U)), ("wv", (C, U)), ("mb", (128, WIN)),
                        ("ident", (128, 128))]:
        aps[name] = nc.dram_tensor(name, list(shape), MM_DT,
                                   kind="ExternalInput").ap()
    for name, shape in [("a_cmp", (NBLK, 128, WIN)), ("out_raw", (U, SH)),
                        ("den", (128, NBLK))]:
        aps[name] = nc.dram_tensor(name, list(shape), F32,
                                   kind="ExternalOutput").ap()
    with tile.TileContext(nc) as tc:
        with ExitStack() as ctx:
            _body(ctx, tc, aps)
    nc.compile()
    return nc


def _get_prog():
    global _PROG
    if _PROG is None:
        _PROG = _build()
    return _PROG


def _band_mask_bias():
    p = np.arange(128)[:, None]
    c = np.arange(WIN)[None, :]
    return np.where((c >= p + 1) & (c <= p + W), np.float32(0.0), NEG
                    ).astype(np.float32)


def _make_in_maps(y1, y2, Wq, Wk, Wv):
    mb = _band_mask_bias()
    ident = np.eye(128, dtype=np.float32)
    wq = (np.asarray(Wq, np.float32) * np.float32(1.0 / np.sqrt(U))).astype(
        np.float32)
    wk = np.ascontiguousarray(np.asarray(Wk, np.float32))
    wv = np.ascontiguousarray(np.asarray(Wv, np.float32))
    in_maps = []
    for core in range(NCORES):
        b, half = divmod(core, 2)
        t0 = half * SH
        y1s = np.ascontiguousarray(y1[b][:, t0:t0 + SH], dtype=np.float32)
        y2s = np.zeros((C, KW), np.float32)
        lo, hi = t0 - HALO, t0 + SH + HALO
        clo, chi = max(lo, 0), min(hi, T)
        y2s[:, clo - lo:chi - lo] = y2[b][:, clo:chi]
        in_maps.append({"y1s": y1s, "y2s": y2s, "wq": wq, "wk": wk, "wv": wv,
                       "mb": mb, "ident": ident})
    return in_maps


def _assemble(results):
    a = np.zeros((B, T, T), np.float32)
    out = np.zeros((B, U, T), np.float32)
    parange = np.arange(128, dtype=np.float32)
    for core in range(NCORES):
        b, half = divmod(core, 2)
        t0 = half * SH
        r = results[core]
        den = np.ascontiguousarray(r["den"].T).reshape(SH).copy()
        if t0 == 0:
            den[0:128] -= np.maximum(0.0, 127.0 - parange).astype(np.float32)
        if t0 + SH == T:
            den[SH - 128:SH] -= (parange + 1.0).astype(np.float32)
        recip = (1.0 / den).astype(np.float32)
        blk = r["a_cmp"]
        for h in range(NBLK):
            j0 = t0 - HALO + 128 * h
            cs, ce = max(0, -j0), min(WIN, T - j0)
            i0 = t0 + 128 * h
            a[b, i0:i0 + 128, j0 + cs:j0 + ce] = (
                blk[h][:, cs:ce] * recip[128 * h:128 * h + 128, None])
        out[b][:, t0:t0 + SH] = r["out_raw"] * recip[None, :]
    return out, a


def _run(y1, y2, Wq, Wk, Wv, **spmd_kwargs):
    nc = _get_prog()
    in_maps = _make_in_maps(y1, y2, Wq, Wk, Wv)
    res = run_bass_kernel_spmd(nc, in_maps, list(range(NCORES)),
                               **spmd_kwargs)
    return res


def kernel(y1, y2, Wq, Wk, Wv, attention_width):
    assert int(attention_width) == W
    y1 = np.asarray(y1, np.float32)
    y2 = np.asarray(y2, np.float32)
    res = _run(y1, y2, Wq, Wk, Wv)
    return _assemble(res.results)


# revision 17
# speedup vs baseline: 1.6032x; 1.6032x over previous
"""Banded cross-attention (sparse_attention) TRN2 Bass kernel.

Problem: q1 = y1^T Wq, k2 = y2^T Wk, v2 = y2^T Wv  (y: [B,C,T], W: [C,U])
         e[b,i,j] = q1[b,i,:].k2[b,j,:]/sqrt(U) masked to band j-i in [-127,128]
         a = softmax(e, axis=-1);  out = einsum('bij,bju->bui', a, v2)
         returns (out [B,U,T], a [B,T,T])

Sharding: 8 cores = (B=4) x (2 halves of T). Each core handles 2048 query rows.

Device per core (SPMD, identical program):
  - projections: qT [64,2048], kT [64,2304] (with 128-halo, zero-padded at
    global edges), v [128,18,64] (18 j-chunks of 128).
  - per 128-row block h (16 blocks): score window [128 x 384]
    (window cols j = t0-128+128h .. +384). PSUM preloaded with additive band
    mask via identity-matmul, then q.k accumulates on top. ACT exp computes
    s = exp(e+mask) and its row-sum (denominator) via accum_out for free.
    s block DMA'd out (unnormalized attention numerators).
  - 3 PE transposes per block build sT [j,i] chunks; out_raw[u,i] += v^T sT
    accumulated over 6 j-chunks per 512-row group.
  - ships: a_cmp [16,128,384], out_raw [64,2048], den [128,16]. All
    unnormalized; host divides by denominators.

Host: corrects denominators at global edges (each in-band slot with padded
k==0 contributed exactly exp(0)=1), normalizes, scatters band into dense a.
"""

import numpy as np
from contextlib import ExitStack

import concourse.bass as bass
import concourse.bacc as bacc
import concourse.tile as tile
from concourse import mybir
from concourse.bass_utils import run_bass_kernel_spmd

F32 = mybir.dt.float32
F32R = mybir.dt.float32r
AF = mybir.ActivationFunctionType
ts = bass.ts

B, C, T, U = 4, 256, 4096, 64
W = 256
NCORES = 8
SH = T // 2             # 2048 rows per core
HALO = 128
KW = SH + 2 * HALO      # 2304 kT columns per core
NBLK = SH // 128        # 16 blocks
NGRP = SH // 512        # 4 groups
WIN = 384               # score window width per block
NEG = np.float32(-1.0e38)

import os as _os
F16 = mybir.dt.float16
# matmul datapath dtype: fp16 (default, 1cyc/col + FWL) or f32r (higher prec)
MM_DT = {"f32r": F32R, "fp16": F16}[_os.environ.get("BANDKERN_DT", "fp16")]
NP_DT = {F32R: np.float32, F16: np.float16}[MM_DT]

# out-matmul emission: (chunk c, col0, col1, start, stop). Chunk c covers
# sT columns written by blocks {c-1,c,c+1} & [0,4). start=True ops together
# first-touch every PSUM column exactly once.
EMITS = [
    (1, 0, 384, True, False),
    (4, 384, 512, False, False),
    (2, 128, 512, False, False),
    (-1, 0, 128, False, False),
    (0, 0, 256, False, False),
    (3, 256, 512, False, True),
]

_PROG = None


def _body(ctx: ExitStack, tc: "tile.TileContext", aps: dict):
    nc = tc.nc

    consts = ctx.enter_context(tc.tile_pool(name="consts", bufs=1))
    proj_sb = ctx.enter_context(tc.tile_pool(name="proj_sb", bufs=1))
    ypool = ctx.enter_context(tc.tile_pool(name="ypool", bufs=5))
    spool = ctx.enter_context(tc.tile_pool(name="spool", bufs=8))
    stpool = ctx.enter_context(tc.tile_pool(name="stpool", bufs=3))

    wc = consts.tile([128, 3 * 2 * U], MM_DT)
    nc.scalar.dma_start(out=wc, in_=aps["wconsts"])
    wq_sb = wc[:, 0:128].rearrange("p (a u) -> p a u", a=2)
    wk_sb = wc[:, 128:256].rearrange("p (a u) -> p a u", a=2)
    wv_sb = wc[:, 256:384].rearrange("p (a u) -> p a u", a=2)
    mident = consts.tile([128, WIN + 128 + WIN], MM_DT)
    nc.gpsimd.dma_start(out=mident, in_=aps["mident"])
    ident = mident[:, WIN:WIN + 128]
    mTs = mident[:, WIN + 128:WIN + 128 + WIN].rearrange(
        "p (q x) -> p q x", q=3)

    qT = proj_sb.tile([64, SH], MM_DT)
    kT = proj_sb.tile([64, KW], MM_DT)
    vsb = proj_sb.tile([128, KW // 128, U + 2], MM_DT)
    nc.gpsimd.dma_start(out=vsb[:, :, U],
                         in_=aps["vones"].rearrange("c p -> p c"))

    y1r = aps["y1s"].rearrange("(a p) t -> p a t", p=128)
    y2r = aps["y2s"].rearrange("(a p) t -> p a t", p=128)

    with tc.tile_pool(name="pps", bufs=2, space="PSUM") as pps:
        # interleaved q/k/v projections; k first so block 0 unblocks early.
        # y2 tiles on the sync HWDGE ring, y1 tiles on the scalar HWDGE ring.
        k_offs = []
        off = 0
        while off < KW:
            k_offs.append(off)
            off += min(512, KW - off)
        steps = []
        for i in range(max(len(k_offs), SH // 512)):
            if i < len(k_offs):
                steps.append(("k", k_offs[i]))
            if i < SH // 512:
                steps.append(("q", i * 512))
        for kind, off in steps:
            if kind == "q":
                it = off // 512
                yt = ypool.tile([128, 2, 512], MM_DT, tag="yt", name="yt")
                nc.gpsimd.dma_start(out=yt[:, 0], in_=y1r[:, 0, ts(it, 512)])
                nc.gpsimd.dma_start(out=yt[:, 1], in_=y1r[:, 1, ts(it, 512)])
                ps = pps.tile([64, 512], F32, tag="pq", name="pq")
                nc.tensor.matmul(ps, wq_sb[:, 0],
                                 yt[:, 0], start=True, stop=False)
                nc.tensor.matmul(ps, wq_sb[:, 1],
                                 yt[:, 1], start=False, stop=True)
                if it % 2 == 0:
                    nc.scalar.copy(qT[:, ts(it, 512)], ps)
                else:
                    nc.vector.tensor_copy(qT[:, ts(it, 512)], ps)
            else:
                n = min(512, KW - off)
                yt = ypool.tile([128, 2, 512], MM_DT, tag="yt", name="yt")
                nc.sync.dma_start(out=yt[:, 0, 0:n], in_=y2r[:, 0, off:off + n])
                nc.sync.dma_start(out=yt[:, 1, 0:n], in_=y2r[:, 1, off:off + n])
                ps = pps.tile([64, 512], F32, tag="pq", name="pq")
                nc.tensor.matmul(ps[:, 0:n], wk_sb[:, 0],
                                 yt[:, 0, 0:n], start=True, stop=False)
                nc.tensor.matmul(ps[:, 0:n], wk_sb[:, 1],
                                 yt[:, 1, 0:n], start=False, stop=True)
                if (off // 512) % 2 == 0:
                    nc.scalar.copy(kT[:, off:off + n], ps[:, 0:n])
                else:
                    nc.vector.tensor_copy(kT[:, off:off + n], ps[:, 0:n])
                # v chunks from this y2 tile
                for cc in range(n // 128):
                    ch = off // 128 + cc
                    pv = pps.tile([128, U], F32, tag="pv", name="pv")
                    nc.tensor.matmul(pv, yt[:, 0, ts(cc, 128)], wv_sb[:, 0],
                                     start=True, stop=False)
                    nc.tensor.matmul(pv, yt[:, 1, ts(cc, 128)], wv_sb[:, 1],
                                     start=False, stop=True)
                    nc.vector.tensor_copy(vsb[:, ch, 0:U], pv)

    eps_pool = ctx.enter_context(tc.tile_pool(name="eps_pool", bufs=4, space="PSUM"))
    trp_pool = ctx.enter_context(tc.tile_pool(name="trp_pool", bufs=2, space="PSUM"))
    ops_pool = ctx.enter_context(tc.tile_pool(name="ops_pool", bufs=2, space="PSUM"))

    for g in range(NGRP):
        sTs = [stpool.tile([128, 512], MM_DT, tag=f"sT{c}", name=f"sT{c}")
               for c in range(6)]
        for hh in range(4):
            h = g * 4 + hh
            eps = eps_pool.tile([128, WIN], F32, tag="eps", name="eps")
            nc.tensor.matmul(eps, qT[:, ts(h, 128)],
                             kT[:, 128 * h:128 * h + WIN],
                             start=True, stop=True)
            s_sb = spool.tile([128, WIN], MM_DT, tag="s", name="s_sb")
            nc.scalar.activation(s_sb, eps, AF.Exp)
            nc.sync.dma_start(out=aps["a_cmp"][h], in_=s_sb)
            trp = trp_pool.tile([128, WIN], MM_DT, tag="trp", name="trp")
            for q3 in range(3):
                nc.tensor.transpose(trp[:, ts(q3, 128)], s_sb[:, ts(q3, 128)],
                                    ident)
            # psum->sbuf copy fused with band masking (mask is pre-transposed)
            for q3 in range(3):
                c = hh + q3 - 1
                nc.vector.tensor_mul(sTs[c + 1][:, ts(hh, 128)],
                                     trp[:, ts(q3, 128)], mTs[:, q3])
        op = ops_pool.tile([U + 1, 512], F32, tag="op", name="op")
        for (c, c0, c1, st, sp) in EMITS:
            vi = 4 * g + c + 1
            nc.tensor.matmul(op[:, c0:c1], vsb[:, vi, 0:U + 1],
                             sTs[c + 1][:, c0:c1],
                             start=st, stop=sp)
        osb = spool.tile([U + 1, 512], F32, tag="osb", name="osb")
        nc.scalar.copy(osb, op)
        nc.sync.dma_start(out=aps["out_raw"][:, ts(g, 512)], in_=osb)



def _build():
    nc = bacc.Bacc("TRN2", target_bir_lowering=False, debug=False,
                   num_devices=NCORES)
    aps = {}
    for name, shape in [("y1s", (C, SH)), ("y2s", (C, KW)),
                        ("wconsts", (128, 3 * 2 * U)),
                        ("mident", (128, WIN + 128 + WIN)),
                        ("vones", (KW // 128, 128))]:
        aps[name] = nc.dram_tensor(name, list(shape), MM_DT,
                                   kind="ExternalInput").ap()
    aps["a_cmp"] = nc.dram_tensor("a_cmp", [NBLK, 128, WIN], MM_DT,
                                  kind="ExternalOutput").ap()
    aps["out_raw"] = nc.dram_tensor("out_raw", [U + 1, SH], F32,
                                    kind="ExternalOutput").ap()
    with tile.TileContext(nc) as tc:
        with ExitStack() as ctx:
            _body(ctx, tc, aps)
    nc.compile()
    return nc


def _get_prog():
    global _PROG
    if _PROG is None:
        _PROG = _build()
    return _PROG


def _band_mask01():
    p = np.arange(128)[:, None]
    c = np.arange(WIN)[None, :]
    return ((c >= p + 1) & (c <= p + W)).astype(np.float32)


_BAND01 = _band_mask01()


def _pack_w(w):
    # [C, U] -> [128, 2, U] with row c = a*128 + p
    return np.ascontiguousarray(
        np.asarray(w, np.float32).reshape(2, 128, U).transpose(1, 0, 2))


def _cvt(x):
    return np.ascontiguousarray(np.asarray(x, np.float32).astype(NP_DT))


def _make_in_maps(y1, y2, Wq, Wk, Wv):
    wconsts = np.zeros((128, 3 * 2 * U), np.float32)
    wq = np.asarray(Wq, np.float32) * np.float32(1.0 / np.sqrt(U))
    wconsts[:, 0:128] = _pack_w(wq).reshape(128, 2 * U)
    wconsts[:, 128:256] = _pack_w(Wk).reshape(128, 2 * U)
    wconsts[:, 256:384] = _pack_w(Wv).reshape(128, 2 * U)
    wconsts = _cvt(wconsts)
    mident = np.zeros((128, WIN + 128 + WIN), np.float32)
    m01 = _band_mask01()
    mident[:, 0:WIN] = m01
    mident[:, WIN:WIN + 128] = np.eye(128, dtype=np.float32)
    for q3 in range(3):
        blkm = m01[:, q3 * 128:(q3 + 1) * 128]
        mident[:, WIN + 128 + q3 * 128:WIN + 256 + q3 * 128] = blkm.T
    mident = _cvt(mident)
    in_maps = []
    for core in range(NCORES):
        b, half = divmod(core, 2)
        t0 = half * SH
        y1s = _cvt(y1[b][:, t0:t0 + SH])
        y2s = np.zeros((C, KW), NP_DT)
        lo, hi = t0 - HALO, t0 + SH + HALO
        clo, chi = max(lo, 0), min(hi, T)
        y2s[:, clo - lo:chi - lo] = _cvt(y2[b][:, clo:chi])
        jglob = (t0 - HALO + np.arange(KW)).reshape(KW // 128, 128)
        vones = ((jglob >= 0) & (jglob < T)).astype(NP_DT)
        in_maps.append({"y1s": y1s, "y2s": y2s, "wconsts": wconsts,
                        "mident": mident, "vones": vones})
    return in_maps


def _assemble(results):
    a = np.zeros((B, T, T), np.float32)
    out = np.zeros((B, U, T), np.float32)
    parange = np.arange(128, dtype=np.float32)
    for core in range(NCORES):
        b, half = divmod(core, 2)
        t0 = half * SH
        r = results[core]
        den = r["out_raw"][U]
        recip = (1.0 / den).astype(np.float32)
        blk = np.asarray(r["a_cmp"], np.float32) * _BAND01[None]
        for h in range(NBLK):
            j0 = t0 - HALO + 128 * h
            cs, ce = max(0, -j0), min(WIN, T - j0)
            i0 = t0 + 128 * h
            a[b, i0:i0 + 128, j0 + cs:j0 + ce] = (
                blk[h][:, cs:ce] * recip[128 * h:128 * h + 128, None])
        out[b][:, t0:t0 + SH] = r["out_raw"][0:U] * recip[None, :]
    return out, a


def _run(y1, y2, Wq, Wk, Wv, **spmd_kwargs):
    nc = _get_prog()
    in_maps = _make_in_maps(y1, y2, Wq, Wk, Wv)
    res = run_bass_kernel_spmd(nc, in_maps, list(range(NCORES)),
                               **spmd_kwargs)
    return res


def kernel(y1, y2, Wq, Wk, Wv, attention_width):
    assert int(attention_width) == W
    y1 = np.asarray(y1, np.float32)
    y2 = np.asarray(y2, np.float32)
    res = _run(y1, y2, Wq, Wk, Wv)
    return _assemble(res.results)
